# revision 46
# baseline (speedup 1.0000x reference)
"""Trainium2 Bass kernel for nn_MultiHeadAttention_56375740727430.

Causal multi-head attention, B=2 S=2048 D=1024 H=16 KS=64, followed by an
output projection `heads @ kernel`.

Sharding: pure data/head parallel over 8 cores - core c handles batch c//4
and 4 heads (c%4)*4 ... +4.  No collectives; the host divides by Z, sums
head contributions and batch-partials, and transposes.

v2 schedule (vs v1's serial projection phase -> attention phase):
- x is DMA'd token-chunk-major (all 8 d-tiles of 512 tokens per chunk), so
  the Q/K projection chunks and V tiles for the first query block complete
  ~4us after the first x chunk lands; attention starts ~15us into the
  kernel instead of ~50us.
- all remaining projection work (Q/K second chunks, V tiles, head-pair-1
  Q/K) is cut into <=2-matmul "filler steps" consumed a few per attention
  l-tile slot, hiding the projection stream inside the softmax-bound
  attention cadence without ever stalling the exp pipeline.
- exp is split across two engines: full l-tiles run on the Scalar/ACT
  engine (exp LUT, free affine scale); diagonal l-tiles run on a custom
  Vector/DVE op EXP8_MASK_ANT computing ((q2(t))^2)^2)^2 * mask - a
  degree-2 minimax polynomial for e^t with three squarings (scores arrive
  pre-scaled by 1/64 via the host-scaled Wq), with the causal mask fused
  as the DVE op's second operand.  This removes the separate mask multiply
  and takes ~30% of the softmax work off the ACT critical path.
- matmul operands are bf16; accumulation fp32.  V carries an appended
  ones-column so the softmax denominator Z falls out of the PV matmul.
- scores pack both heads of a pair via row tile_position (concurrent);
  the output projection packs both heads block-diagonally.
"""

import re
import sys

sys.path.insert(0, "/opt/trn_rl_repo")

from contextlib import ExitStack

import ml_dtypes
import numpy as np

import concourse.bass as bass
import concourse.bacc as bacc
import concourse.mybir as mybir
import concourse.tile as tile

B, S, D = 2, 2048, 1024
H, KS = 16, 64

P = 128            # partitions
NCORES = 8
CORES_PER_B = NCORES // B          # 4
NH = H // CORES_PER_B              # heads per core = 4
NW = NH * KS                       # per-core projection width = 256
DT = D // P                        # d-tiles = 8
ST = S // P                        # s/l-tiles = 16
IB = 512                           # query block
NIB = S // IB                      # 4
LPB = IB // P                      # l-tiles per query block = 4

F32 = mybir.dt.float32
BF16 = mybir.dt.bfloat16
NP_BF16 = ml_dtypes.bfloat16
EXP = mybir.ActivationFunctionType.Exp

# degree-2 minimax fit of e^t on [-0.375, 0.375] with intercept pinned at 1;
# the DVE op computes (((p)^2)^2)^2 = exp(8t) for t = score/64.
C_EXP_LIN = 1.0164435232954696
C_EXP_QUAD = 0.4989002612952626

FILLER_STEPS_PER_SLOT = 4
DVE_EXP = True  # debug flag: False = all exp on ACT + separate mask mul
ALT_EXP = False  # alternate exp engine by slot parity (False: diag->DVE only)
DEBUG_DUMP = False


def _exp8_ref(in0, in1, c0, c1, c2):
    t = np.asarray(in0, np.float32)
    p = (np.float32(c1) * t + np.float32(c0)) * t + np.float32(1.0)
    r = ((p * p) ** 2) ** 2
    if in1 is not None:
        m = np.asarray(in1, np.float32)
        if m.size == t.shape[0]:
            m = m.reshape((t.shape[0],) + (1,) * (t.ndim - 1))
        else:
            m = m.reshape(t.shape)
        r = r * m
    return r.astype(np.float32)


_EXP8_OP = None


def get_exp8_op():
    """Register the EXP8_MASK_ANT custom DVE op (idempotent)."""
    global _EXP8_OP
    if _EXP8_OP is not None:
        return _EXP8_OP
    from concourse import dve_ops
    from concourse.dve_spec import C0, C1, One, Spec, Src0, Src1, lower, sq
    from concourse.dve_uop import DveOpSpec

    name = "EXP8_MASK_ANT"
    if name in dve_ops._SUB_OPCODE_FOR_NAME:
        for op in dve_ops.OPS:
            if op.name == name:
                _EXP8_OP = op
                return op
    spec = Spec(
        body=sq(sq(sq((C1 * Src0 + C0) * Src0 + One))) * Src1,
        reference=_exp8_ref,
    )
    row = max(dve_ops._SUB_OPCODE_FOR_NAME.values()) + 1
    assert row < 0x20
    dve_ops._SUB_OPCODE_FOR_NAME[name] = row
    shas = {}
    for ver in ("v3", "v4"):
        shas[ver] = DveOpSpec(
            name=name,
            opcode=row,
            uops=lower(spec, ver=ver),
            rd1_en=True,
        ).sha(ver)
    op = dve_ops.DveOp(name, spec, subdim=False, uops_sha=shas)
    dve_ops.OPS.append(op)
    dve_ops.CUSTOM_DVE_SPECS[name] = spec
    _EXP8_OP = op
    return op


def build_nc():
    mm_dt = BF16
    exp8 = get_exp8_op()
    nc = bacc.Bacc()

    # host-pre-shuffled layouts (partition-major, chunk-contiguous)
    xT = nc.declare_dram_parameter("xT", [P, NIB, DT, IB], mm_dt, isOutput=False)
    wq = nc.declare_dram_parameter("wq", [P, 2, DT, P], mm_dt, isOutput=False)
    wk = nc.declare_dram_parameter("wk", [P, 2, DT, P], mm_dt, isOutput=False)
    wv = nc.declare_dram_parameter("wv", [P, DT, NW], mm_dt, isOutput=False)
    # pair layout: partition = hh*KS + k, dims = [head pair, j]
    wkern = nc.declare_dram_parameter("wkern", [P, NH // 2, KS], mm_dt, isOutput=False)
    # [P, 2, IB]: triangular strip (128 cols) then ones
    masks = nc.declare_dram_parameter("masks", [P, 2, IB], mm_dt, isOutput=False)
    outT = nc.declare_dram_parameter("outT", [KS, NIB, NH, IB], mm_dt, isOutput=True)
    # Z partials: per (head pair, block), rows 0/32/64/96 hold the four
    # (l-parity, head) partial sums; the host adds parities.
    z = nc.declare_dram_parameter("z", [2, NIB, P, IB], F32, isOutput=True)
    if DEBUG_DUMP:
        dbg_qt = nc.declare_dram_parameter("dbg_qt", [2, P, S], mm_dt, isOutput=True)
        dbg_kt = nc.declare_dram_parameter("dbg_kt", [2, P, S], mm_dt, isOutput=True)
        dbg_v = nc.declare_dram_parameter(
            "dbg_v", [P, ST, NH, KS], mm_dt, isOutput=True
        )

    with tile.TileContext(nc) as tc, ExitStack() as ctx:
        const_pool = ctx.enter_context(tc.tile_pool(name="const", bufs=1))
        qkv_pool = ctx.enter_context(tc.tile_pool(name="qkv", bufs=1))
        out_pool = ctx.enter_context(tc.tile_pool(name="outp", bufs=1))
        xw_pool = ctx.enter_context(tc.tile_pool(name="xw", bufs=1))
        pexp_pool = ctx.enter_context(tc.tile_pool(name="pexp", bufs=6))
        osb_pool = ctx.enter_context(tc.tile_pool(name="osb", bufs=4))

        ones1 = const_pool.tile([P, 1], F32)
        nc.gpsimd.memset(ones1[:], 1.0)
        # touch Exp so the ~2.7us ACT_TABLE_LOAD doesn't land on the first
        # real softmax tile
        warm_exp = const_pool.tile([P, 1], F32)
        nc.scalar.activation(warm_exp[:], ones1[:], EXP)

        qt_sb = [
            qkv_pool.tile([P, S], mm_dt, tag=f"qt{i}", name=f"qt{i}") for i in range(2)
        ]
        kt_sb = [
            qkv_pool.tile([P, S], mm_dt, tag=f"kt{i}", name=f"kt{i}") for i in range(2)
        ]
        v_sb = qkv_pool.tile([P, ST, NH, KS], mm_dt, tag="v")
        ones_col = const_pool.tile([P, 32], mm_dt)
        nc.gpsimd.memset(ones_col[:], 1.0)
        ones_full = const_pool.tile([P, 2, IB], mm_dt)
        nc.gpsimd.memset(ones_full[:], 1.0)
        outT_sb = out_pool.tile([KS, NH, S], mm_dt)

        # input DMA: the half-weights + first x chunk gate the attention
        # start; everything else streams behind them.
        w_sb = {}
        for wn, wh in (("q", wq), ("k", wk)):
            w_sb[wn] = xw_pool.tile([P, 2, DT, P], mm_dt, tag=f"w{wn}", name=f"w{wn}")
        wv_sb = xw_pool.tile([P, DT, NW], mm_dt, tag="wv", name="wv")
        xT_sb = xw_pool.tile([P, NIB, DT, IB], mm_dt, tag="xT")
        wkern_sb = const_pool.tile([P, NH // 2, KS], mm_dt)
        mask_sb = const_pool.tile([P, 2, IB], mm_dt)

        # Three parallel DMA initiators (Sync + Scalar HWDGE, gpsimd SWDGE);
        # per-queue effective bandwidth is only ~120-190 GB/s, so the 5.75MB
        # of input is spread so that the block-(0,0) set (wk0/wq0/xc0/wv/
        # masks) all lands ~12-13us in.
        nc.sync.dma_start(xT_sb[:, 0], xT[:, 0])
        nc.sync.dma_start(xT_sb[:, 1, 0:4], xT[:, 1, 0:4])
        nc.sync.dma_start(xT_sb[:, 2], xT[:, 2])
        nc.sync.dma_start(xT_sb[:, 3], xT[:, 3])
        nc.scalar.dma_start(w_sb["k"][:, 0], wk[:, 0])
        nc.scalar.dma_start(w_sb["q"][:, 0], wq[:, 0])
        nc.scalar.dma_start(xT_sb[:, 1, 4:8], xT[:, 1, 4:8])
        nc.scalar.dma_start(wkern_sb[:], wkern[:])
        nc.gpsimd.dma_start(mask_sb[:], masks[:])
        nc.gpsimd.dma_start(wv_sb[:], wv[:])
        nc.gpsimd.dma_start(w_sb["k"][:, 1], wk[:, 1])
        nc.gpsimd.dma_start(w_sb["q"][:, 1], wq[:, 1])

        pst = ctx.enter_context(
            tc.tile_pool(name="pst", bufs=2, space=bass.MemorySpace.PSUM)
        )
        po = ctx.enter_context(
            tc.tile_pool(name="po", bufs=4, space=bass.MemorySpace.PSUM)
        )

        # PE warm-up during the input-DMA wait: ~10 matmuls on a zeroed
        # tile flip the HAM clock gate to 2.4 GHz right as the first x
        # chunk lands, so the real stream never runs throttled.
        warm_in = const_pool.tile([P, IB], mm_dt)
        nc.vector.memset(warm_in[:], 0.0)
        for _ in range(10):
            w_ps = po.tile([P, IB], F32, tag="of", name="w_ps")
            nc.tensor.matmul(
                w_ps[:], warm_in[:, 0:P], warm_in[:], start=True, stop=True
            )

        # ---- projection chunks, emitted either inline (pre-phase) or as
        # fine-grained filler steps inside the attention loop.
        def qk_chunk_steps(wn, half, ic):
            state = {}

            def start():
                state["ps"] = po.tile([P, IB], F32, tag="of", name=f"qk{half}_{wn}{ic}")

            def mm_pair(tp):
                def step():
                    if tp == 0:
                        start()
                    for t in (2 * tp, 2 * tp + 1):
                        nc.tensor.matmul(
                            state["ps"][:],
                            w_sb[wn][:, half, t, :],
                            xT_sb[:, ic, t, :],
                            start=(t == 0),
                            stop=(t == DT - 1),
                        )
                return step

            def drain():
                dst = qt_sb if wn == "q" else kt_sb
                nc.vector.tensor_copy(
                    dst[half][:, ic * IB : (ic + 1) * IB], state["ps"][:]
                )

            return [mm_pair(tp) for tp in range(DT // 2)] + [drain]

        def v_chunk_steps(st):
            state = {}

            def mm_pair(tp):
                def step():
                    if tp == 0:
                        state["ps"] = po.tile([P, NW], F32, tag="of", name=f"v{st}")
                    for t in (2 * tp, 2 * tp + 1):
                        nc.tensor.matmul(
                            state["ps"][:],
                            xT_sb[:, st // LPB, t, (st % LPB) * P : (st % LPB + 1) * P],
                            wv_sb[:, t, :],
                            start=(t == 0),
                            stop=(t == DT - 1),
                        )
                return step

            def drain():
                nc.vector.tensor_copy(
                    v_sb[:, st, :, :],
                    state["ps"][:].rearrange("p (h k) -> p h k", k=KS),
                )

            return [mm_pair(tp) for tp in range(DT // 2)] + [drain]

        # pre-phase: minimal set for block (0,0)
        for steps in (
            qk_chunk_steps("k", 0, 0),
            qk_chunk_steps("q", 0, 0),
            v_chunk_steps(0),
            v_chunk_steps(1),
            v_chunk_steps(2),
            v_chunk_steps(3),
        ):
            for s in steps:
                s()

        # filler queue: remaining projection chunks, each step tagged with
        # the attention slot by which it must be done (EDF consumption:
        # 2 steps/slot baseline, more when a deadline approaches).
        blocks = [(0, 0), (0, 1), (0, 2), (0, 3), (1, 0), (1, 1), (1, 2), (1, 3)]
        block_start = {}
        s = 0
        for pr, ib in blocks:
            block_start[(pr, ib)] = s
            s += (ib + 1) * LPB
        fillers = []
        for half in range(2):
            for ic in range(NIB):
                if half == 0 and ic == 0:
                    continue  # pre-phase
                dl_q = block_start[(half, ic)] - 1
                dl_k = block_start[(half, ic)] + ic * LPB - 1
                for st in qk_chunk_steps("q", half, ic):
                    fillers.append((dl_q, st))
                for st in qk_chunk_steps("k", half, ic):
                    fillers.append((dl_k, st))
        for vst in range(4, ST):
            dl = block_start[(0, vst // LPB)] + vst
            for st in v_chunk_steps(vst):
                fillers.append((dl, st))
        fillers.sort(key=lambda x: x[0])

        pending_end = [None, None]
        slot_ctr = [0]

        def attention(blocks):
            # causal attention + output projection, one (head pair, query
            # block) at a time; scores row-packed via tile_position so both
            # heads' K=64 matmuls share the PE array.  Inner loop is
            # software-pipelined: PV(lt) is emitted after exp(lt+1)'s input
            # so the PE isn't gated on exp, and each block's output
            # projection is deferred into the next block.
            for pr, ib in blocks:
                nl = (ib + 1) * LPB
                # both heads' PV column-packed into one PSUM tile (rows
                # 0-63 = head 0, 64-127 = head 1), streamed concurrently;
                # Z comes from a separate 4-way column-tiled ones pass
                # every two l-tiles (one stream-N for 2 l-tiles x 2 heads).
                o_ps = po.tile([P, IB], F32, tag="of", name=f"o{pr}_{ib}")
                zps_box = {}
                seen = {}

                def emit_pv(lt, pe, off, o_ps=o_ps, zps_box=zps_box,
                            seen=seen, pr=pr, nl=nl):
                    seen[lt] = (pe, off)
                    for hh in range(2):
                        # per-column-tile accumulation groups; the sim's
                        # group tracker is partition-base-blind, so skip it
                        # (start/stop zeroing semantics are per partition
                        # range - cf. v1's block-diagonal fproj)
                        nc.tensor.matmul(
                            o_ps[hh * KS : (hh + 1) * KS, off:IB],
                            v_sb[:, lt, 2 * pr + hh, :],
                            pe[:, hh, off:IB],
                            start=(lt == 0),
                            stop=(lt == nl - 1),
                            tile_position=(0, hh * KS),
                            skip_group_check=(hh == 1),
                        )
                    if lt % 2 == 1:
                        all_diag = nl == LPB  # ib == 0: no full-width pair
                        if "z" not in zps_box:
                            zps_box["z"] = po.tile(
                                [P, IB], F32, tag="of", name=f"z{pr}"
                            )
                            if all_diag:
                                # zero-fill the bank so the block-end copy
                                # never reads bytes no Z tile wrote
                                nc.tensor.matmul(
                                    zps_box["z"][:, :],
                                    warm_in[:, 0:P],
                                    warm_in[:, :],
                                    start=True,
                                    stop=True,
                                )
                        z_ps = zps_box["z"]
                        m = lt // 2
                        for parity in (0, 1):
                            zpe, zoff = seen[lt - 1 + parity]
                            for hh in range(2):
                                col = 32 * (2 * parity + hh)
                                nc.tensor.matmul(
                                    z_ps[col : col + 32, zoff:IB],
                                    ones_col[:, :],
                                    zpe[:, hh, zoff:IB],
                                    start=(m == 0 and not all_diag),
                                    stop=(m == nl // 2 - 1),
                                    tile_position=(0, col),
                                    skip_group_check=(col > 0 or all_diag),
                                )
                        del seen[lt - 1], seen[lt]

                pending = []
                for lt in range(nl):
                    # causal: columns [0, off) of this i-block are fully
                    # masked for key tile lt; compute only the suffix
                    off = max(0, (lt - ib * LPB)) * P
                    st_ps = pst.tile([P, 2, IB], F32, tag="st", name="st")
                    for hh in range(2):
                        nc.tensor.matmul(
                            st_ps[:, hh, off:IB],
                            kt_sb[pr][hh * KS : (hh + 1) * KS, lt * P : (lt + 1) * P],
                            qt_sb[pr][
                                hh * KS : (hh + 1) * KS,
                                ib * IB + off : (ib + 1) * IB,
                            ],
                            start=True,
                            stop=True,
                            tile_position=(hh * KS, 0),
                        )
                    slot = slot_ctr[0]
                    slot_ctr[0] += 1
                    pe = pexp_pool.tile([P, 2, IB], BF16, tag="pe", name="pe")
                    is_diag = lt >= ib * LPB
                    # alternate the exp between ACT and DVE by slot parity
                    # so neither engine serializes a stretch of slots; the
                    # ACT diagonal path gets its causal mask from the
                    # otherwise-idle gpsimd engine.
                    use_dve = DVE_EXP and (
                        (slot % 2 == 1) if ALT_EXP else is_diag
                    )
                    if use_dve:
                        nc.vector._custom_dve(
                            exp8,
                            out=pe[:, :, off:IB],
                            in0=st_ps[:, :, off:IB],
                            in1=(
                                mask_sb[:, :, 0 : IB - off]
                                if is_diag
                                else ones_full[:, :, :]
                            ),
                            s0=C_EXP_LIN,
                            s1=C_EXP_QUAD,
                        )
                    else:
                        nc.scalar.activation(
                            pe[:, :, off:IB], st_ps[:, :, off:IB], EXP, scale=8.0
                        )
                        if is_diag:
                            nc.vector.tensor_mul(
                                pe[:, :, off : off + P],
                                pe[:, :, off : off + P],
                                mask_sb[:, :, 0:P],
                            )
                    if len(pending) >= 2:
                        emit_pv(*pending.pop(0))
                    if lt == 1 and pending_end[0] is not None:
                        pending_end[1] = pending_end[0]()  # frees o_ps banks
                        pending_end[0] = None
                    if lt == 2 and pending_end[1] is not None:
                        pending_end[1]()  # projection matmuls + output DMA
                        pending_end[1] = None
                    taken = 0
                    while fillers and (
                        taken < 1 or fillers[0][0] <= slot + 3
                    ):
                        fillers.pop(0)[1]()
                        taken += 1
                        if taken >= 6:
                            break
                    pending.append((lt, pe, off))
                while pending:
                    emit_pv(*pending.pop(0))
                pending_end[0] = (
                    lambda pr=pr, ib=ib, o_ps=o_ps, zps_box=zps_box: emit_ib_end(
                        pr, ib, o_ps, zps_box["z"]
                    )
                )

        def emit_ib_end(pr, ib, o_ps, z_ps):
            # part A: drain o/z out of PSUM (frees the accumulator banks);
            # returns part B: the block-diagonal projection pair + output DMA
            o_bf = osb_pool.tile([P, IB], BF16, tag="o_bf", name="o_bf")
            nc.vector.tensor_copy(o_bf[:], o_ps[:])
            z_sb = osb_pool.tile([P, IB], F32, tag="z_sb", name="z_sb")
            nc.scalar.copy(z_sb[:], z_ps[:])
            nc.sync.dma_start(z[pr, ib], z_sb[:])

            def part_b():
                f_ps = po.tile([P, IB], F32, tag="of", name="f_ps")
                for hh in range(2):
                    nc.tensor.matmul(
                        f_ps[hh * KS : (hh + 1) * KS, :],
                        wkern_sb[hh * KS : (hh + 1) * KS, pr, :],
                        o_bf[hh * KS : (hh + 1) * KS, :],
                        start=True,
                        stop=True,
                        tile_position=(hh * KS, hh * KS),
                    )
                for hh in range(2):
                    h = 2 * pr + hh
                    nc.vector.tensor_copy(
                        outT_sb[:, h, ib * IB : (ib + 1) * IB],
                        f_ps[hh * KS : (hh + 1) * KS, :],
                    )
                nc.sync.dma_start(
                    outT[:, ib, 2 * pr : 2 * pr + 2, :],
                    outT_sb[:, 2 * pr : 2 * pr + 2, ib * IB : (ib + 1) * IB],
                )

            return part_b

        attention(blocks)
        while fillers:  # safety: shouldn't trigger
            fillers.pop(0)[1]()
        if pending_end[0] is not None:  # final query block: drain + project
            pending_end[1] = pending_end[0]()
        if pending_end[1] is not None:
            pending_end[1]()
        if DEBUG_DUMP:
            for half in range(2):
                nc.sync.dma_start(dbg_qt[half], qt_sb[half][:])
                nc.sync.dma_start(dbg_kt[half], kt_sb[half][:])
            nc.sync.dma_start(dbg_v[:], v_sb[:])

    nc.compile()
    return nc


def make_masks():
    # [P, 2, IB]: triangular strip (keep j >= p) then ones
    j = np.arange(IB)[None, :]
    p = np.arange(P)[:, None]
    m = ((j >= p) | (j >= P)).astype(NP_BF16)
    return np.stack([m, m], axis=1)  # [P, 2, IB]


def make_in_maps(inputs):
    x = np.asarray(inputs["x"], np.float32)
    Wq = np.asarray(inputs["Wq"], np.float32)
    Wk = np.asarray(inputs["Wk"], np.float32)
    Wv = np.asarray(inputs["Wv"], np.float32)
    kern = np.asarray(inputs["kernel"], np.float32)

    masks = make_masks()
    kern3 = kern.reshape(KS, H, KS)  # [k, h, j]
    in_maps = []
    for c in range(NCORES):
        b, hs = c // CORES_PER_B, (c % CORES_PER_B) * NH
        # wkern pair layout: [hh*KS + k, pair, j] for heads h = hs + 2*pair + hh
        kern_c = kern3[:, hs : hs + NH, :]  # [k, h, j]
        wkern_pair = np.zeros((P, NH // 2, KS), np.float32)
        for pair in range(NH // 2):
            for hh in range(2):
                wkern_pair[hh * KS : (hh + 1) * KS, pair, :] = kern_c[
                    :, 2 * pair + hh, :
                ]

        def qk_lay(W, scale=1.0):
            # [D, KS, NH-slice] -> [P(d-part), 2(half), DT, P(=2 heads*KS)]
            w = (W[:, :, hs : hs + NH] * scale).transpose(0, 2, 1)  # [D, NH, KS]
            w = w.reshape(DT, P, 2, P)  # [DT, P(dpart), half, 128]
            return w.transpose(1, 2, 0, 3).astype(NP_BF16)  # [P, 2, DT, P]

        xb = x[b].T.reshape(DT, P, NIB, IB)  # [DT, P, NIB, IB]
        in_maps.append(
            {
                "xT": xb.transpose(1, 2, 0, 3).astype(NP_BF16),  # [P, NIB, DT, IB]
                "wq": qk_lay(Wq, 1.0 / 64.0),
                "wk": qk_lay(Wk),
                "wv": Wv[:, :, hs : hs + NH].transpose(0, 2, 1).reshape(DT, P, NW)
                .transpose(1, 0, 2).astype(NP_BF16),
                "wkern": wkern_pair.astype(NP_BF16),
                "masks": masks,
            }
        )
    return in_maps


def reduce_z(zd):
    """[2, NIB, P, IB] partials -> [NH, S]: rows 0+64 = head 2*pr,
    rows 32+96 = head 2*pr+1."""
    zd = np.asarray(zd, np.float32)
    zz = np.zeros((NH, S), np.float32)
    for pr in range(2):
        for ib in range(NIB):
            blk = zd[pr, ib]
            zz[2 * pr, ib * IB : (ib + 1) * IB] = blk[0] + blk[64]
            zz[2 * pr + 1, ib * IB : (ib + 1) * IB] = blk[32] + blk[96]
    return zz


def gather_output(results):
    out = np.zeros((B, S, KS), np.float32)
    for c in range(NCORES):
        b = c // CORES_PER_B
        oT = (
            np.asarray(results[c]["outT"], np.float32)  # [KS, NIB, NH, IB]
            .transpose(2, 0, 1, 3)
            .reshape(NH, KS, S)
        )
        zz = reduce_z(results[c]["z"])
        out[b] += (oT / zz[:, None, :]).sum(axis=0).T
    return out


_NC_CACHE = {}


def get_nc():
    if "nc" not in _NC_CACHE:
        _NC_CACHE["nc"] = build_nc()
    return _NC_CACHE["nc"]


def run_hw(inputs, trace=False, **kw):
    from concourse.bass_utils import run_bass_kernel_spmd

    nc = get_nc()
    in_maps = make_in_maps(inputs)
    res = run_bass_kernel_spmd(
        nc, in_maps, list(range(NCORES)), trace=trace, **kw
    )
    return gather_output(res.results), res


def kernel(**inputs) -> np.ndarray:
    out, _ = run_hw(inputs, trace=False)
    return out
